# revision 4
# baseline (speedup 1.0000x reference)
"""Trainium2 Bass kernel for nn_Discriminator (W_down projection + time-embed
+ W_vt/W_ih projections + 16-step LSTM + linear head).

Strategy: pure data-parallel over batch B=128 across 8 NeuronCores (16
batches/core), all weights replicated. Heavy operands are pre-transposed,
padded, and cast to fp16 on host (layout/precision prep only — all contractions
run on-device with fp32 PSUM accumulation). Everything on-device is laid out
feature-on-partition ("T layout") so the LSTM elementwise ops run on full 128
partitions:

  vT[n, r]      : [20096, 256]  r = l*16 + b (l-major rows per core)
  vdT = WdT.T @ vT   (contraction n, accumulated over 157 K-tiles)
  teT = wt (x) t + wtb (rank-1 matmuls, K=1)
  inputsT = WvtT.T @ [vdT; teT]
  xgT = WihT.T @ inputsT + (b_ih + b_hh)      [4096, 256] fp16, SBUF resident
  LSTM (weight-stationary, gates transposed):
     gatesT_t = xgT[:, t] + sum_k WhhT[k].T @ hT[k]   in two [128,256] PSUM halves
     i,f,g,o on [128,128] tiles; cT/hT as [128, 8*16]
  pred = hT.T @ lin_wT + lin_b  -> [16, 1] per core
"""

import numpy as np

import concourse.bass as bass
import concourse.bacc as bacc
import concourse.tile as tile
from concourse import mybir
from concourse.bass_utils import run_bass_kernel_spmd

F32 = mybir.dt.float32
F16 = mybir.dt.float16

B, L, N = 128, 16, 20000
HIN, H, HT = 512, 1024, 128
G = 4 * H                     # 4096 gate rows
NCORES = 8
BLOC = B // NCORES            # 16 batches per core
R = BLOC * L                  # 256 rows per core (l-major)
P = 128
NT = (N + P - 1) // P         # 157 contraction tiles
NPAD = NT * P                 # 20096
DT = HIN // P                 # 4 vd tiles
KVT = (HIN + HT) // P         # 5 vt contraction tiles
MC = G // P                   # 32 gate row-tiles
KC = HIN // P                 # 4 xg contraction tiles
KH = H // P                   # 8 lstm contraction tiles

_CACHE = {}


def _build():
    """Build + compile the SPMD Bass module once."""
    if "nc" in _CACHE:
        return _CACHE["nc"]

    nc = bacc.Bacc("TRN2", target_bir_lowering=False, debug=False,
                   num_devices=NCORES)

    d_vT = nc.dram_tensor("vT", [NPAD, R], F16, kind="ExternalInput")
    d_WdT = nc.dram_tensor("WdT", [NPAD, HIN], F16, kind="ExternalInput")
    d_WvtT = nc.dram_tensor("WvtT", [HIN + HT, HIN], F16, kind="ExternalInput")
    d_WihT = nc.dram_tensor("WihT", [HIN, G], F16, kind="ExternalInput")
    d_WhhT = nc.dram_tensor("WhhT", [H, G], F16, kind="ExternalInput")
    d_t = nc.dram_tensor("t_row", [1, R], F32, kind="ExternalInput")
    d_wt = nc.dram_tensor("wt_row", [1, HT], F32, kind="ExternalInput")
    d_wtb = nc.dram_tensor("wtb_row", [1, HT], F32, kind="ExternalInput")
    d_bias = nc.dram_tensor("bias_g", [P, MC], F32, kind="ExternalInput")
    d_linw = nc.dram_tensor("lin_wT", [P, KH], F16, kind="ExternalInput")
    d_linb = nc.dram_tensor("lin_b_col", [BLOC, 1], F32, kind="ExternalInput")
    d_pred = nc.dram_tensor("pred", [BLOC, 1], F32, kind="ExternalOutput")

    SIG = mybir.ActivationFunctionType.Sigmoid
    TANH = mybir.ActivationFunctionType.Tanh
    IDENT = mybir.ActivationFunctionType.Identity

    with tile.TileContext(nc) as tc:
        with (
            tc.tile_pool(name="const", bufs=1) as const,
            tc.tile_pool(name="vstream", bufs=4) as vpool,
            tc.tile_pool(name="wdstream", bufs=4) as wdpool,
            tc.tile_pool(name="ws", bufs=2) as ws,
            tc.tile_pool(name="h16", bufs=2) as h16pool,
            tc.tile_pool(name="psmm", bufs=1, space="PSUM") as psmm,
        ):
            # ---- resident weights / constants ----
            whh_sb = const.tile([P, KH * G], F16)      # 64KB/part
            for k in range(KH):
                nc.sync.dma_start(out=whh_sb[:, k * G:(k + 1) * G],
                                  in_=d_WhhT[k * P:(k + 1) * P, :])
            wih_sb = const.tile([P, KC * G], F16)      # 32KB/part
            for k in range(KC):
                nc.sync.dma_start(out=wih_sb[:, k * G:(k + 1) * G],
                                  in_=d_WihT[k * P:(k + 1) * P, :])
            wvt_sb = const.tile([P, KVT * HIN], F16)   # 5KB/part
            for k in range(KVT):
                nc.sync.dma_start(out=wvt_sb[:, k * HIN:(k + 1) * HIN],
                                  in_=d_WvtT[k * P:(k + 1) * P, :])
            bias_sb = const.tile([P, MC], F32)
            nc.sync.dma_start(out=bias_sb, in_=d_bias[:])
            linw_sb = const.tile([P, KH], F16)
            nc.sync.dma_start(out=linw_sb, in_=d_linw[:])
            linb_sb = const.tile([BLOC, 1], F32)
            nc.sync.dma_start(out=linb_sb, in_=d_linb[:])
            t_sb = const.tile([1, R], F32)
            nc.sync.dma_start(out=t_sb, in_=d_t[:])
            wt_sb = const.tile([1, HT], F32)
            nc.sync.dma_start(out=wt_sb, in_=d_wt[:])
            wtb_sb = const.tile([1, HT], F32)
            nc.sync.dma_start(out=wtb_sb, in_=d_wtb[:])
            ones_sb = const.tile([1, R], F32)
            nc.vector.memset(ones_sb, 1.0)

            vt_sb = const.tile([P, KVT * R], F16)      # vdT + teT
            inpT_sb = const.tile([P, KC * R], F16)     # inputsT
            xgT_sb = const.tile([P, MC * R], F16)      # 16KB/part
            cT = const.tile([P, HT], F32)              # cell state [128, 8*16]

            # ---- phase A: vdT[d, r] += WdT[n,d].T @ vT[n,r] ----
            psA = [psmm.tile([P, R], F32, tag=f"psA{d}", name=f"psA{d}")
                   for d in range(DT)]
            for n in range(NT):
                wd_t = wdpool.tile([P, HIN], F16, tag="wd", name="wd_t", bufs=4)
                nc.sync.dma_start(out=wd_t, in_=d_WdT[n * P:(n + 1) * P, :])
                v_t = vpool.tile([P, R], F16, tag="v", name="v_t", bufs=4)
                nc.sync.dma_start(out=v_t, in_=d_vT[n * P:(n + 1) * P, :])
                for d in range(DT):
                    nc.tensor.matmul(psA[d], lhsT=wd_t[:, d * P:(d + 1) * P],
                                     rhs=v_t, start=(n == 0), stop=(n == NT - 1))
            # te: wt[j] * t[r] + wtb[j]  (rank-1 updates, K=1) — reuses an
            # LSTM psum slot (free during the front phases)
            psTE = psmm.tile([P, R], F32, tag="psL0", name="psTE", bufs=2)
            nc.tensor.matmul(psTE, lhsT=wt_sb, rhs=t_sb, start=True, stop=False)
            nc.tensor.matmul(psTE, lhsT=wtb_sb, rhs=ones_sb, start=False,
                             stop=True)
            for d in range(DT):
                if d % 2 == 0:
                    nc.vector.tensor_copy(out=vt_sb[:, d * R:(d + 1) * R],
                                          in_=psA[d])
                else:
                    nc.scalar.copy(out=vt_sb[:, d * R:(d + 1) * R], in_=psA[d])
            nc.vector.tensor_copy(out=vt_sb[:, DT * R:(DT + 1) * R], in_=psTE)

            # ---- phase B: inputsT[i, r] ----
            for m in range(KC):
                psB = psmm.tile([P, R], F32, tag=f"psA{m}", name=f"psB{m}")
                for k in range(KVT):
                    nc.tensor.matmul(
                        psB,
                        lhsT=wvt_sb[:, k * HIN + m * P:k * HIN + (m + 1) * P],
                        rhs=vt_sb[:, k * R:(k + 1) * R],
                        start=(k == 0), stop=(k == KVT - 1))
                nc.vector.tensor_copy(out=inpT_sb[:, m * R:(m + 1) * R],
                                      in_=psB)

            # ---- phase C: xgT[g, r] = WihT.T @ inputsT + bias ----
            for m in range(MC):
                psC = psmm.tile([P, R], F32, tag=f"psA{m % DT}", name=f"psC{m}")
                for k in range(KC):
                    nc.tensor.matmul(
                        psC,
                        lhsT=wih_sb[:, k * G + m * P:k * G + (m + 1) * P],
                        rhs=inpT_sb[:, k * R:(k + 1) * R],
                        start=(k == 0), stop=(k == KC - 1))
                nc.scalar.activation(out=xgT_sb[:, m * R:(m + 1) * R], in_=psC,
                                     func=IDENT, bias=bias_sb[:, m:m + 1],
                                     scale=1.0)

            # ---- LSTM over L steps ----
            xg3 = xgT_sb.rearrange("p (m r) -> p m r", m=MC)   # [128, 32, 256]
            h_prev = None
            for t in range(L):
                acts = []
                if t == 0:
                    for g in range(4):
                        a = ws.tile([P, HT], F32, tag=f"act{g}",
                                    name=f"act{g}_{t}")
                        a3 = a.rearrange("p (m j) -> p m j", m=KH)
                        nc.scalar.activation(
                            out=a3, in_=xg3[:, g * KH:(g + 1) * KH,
                                            t * BLOC:(t + 1) * BLOC],
                            func=(TANH if g == 2 else SIG))
                        acts.append(a)
                else:
                    # two psum halves: (i,f) and (g,o) so DVE/ACT overlap PE
                    pshs = [psmm.tile([P, 2 * HT], F32, tag=f"psL{half}",
                                      name=f"psL{half}_{t}", bufs=2)
                            for half in range(2)]
                    for m in range(MC):
                        half, mm = divmod(m, MC // 2)
                        ph = pshs[half]
                        for k in range(KH):
                            nc.tensor.matmul(
                                ph[:, mm * BLOC:(mm + 1) * BLOC],
                                lhsT=whh_sb[:, k * G + m * P:k * G + (m + 1) * P],
                                rhs=h_prev[:, k * BLOC:(k + 1) * BLOC],
                                start=(k == 0), stop=(k == KH - 1))
                    for g in range(4):
                        ph3 = pshs[g // 2].rearrange("p (m j) -> p m j",
                                                     m=MC // 2)
                        pre = ws.tile([P, HT], F32, tag=f"pre{g}",
                                      name=f"pre{g}_{t}")
                        pre3 = pre.rearrange("p (m j) -> p m j", m=KH)
                        nc.vector.tensor_add(
                            pre3,
                            ph3[:, (g % 2) * KH:(g % 2 + 1) * KH, :],
                            xg3[:, g * KH:(g + 1) * KH,
                                t * BLOC:(t + 1) * BLOC])
                        a = ws.tile([P, HT], F32, tag=f"act{g}",
                                    name=f"act{g}_{t}")
                        nc.scalar.activation(out=a, in_=pre,
                                             func=(TANH if g == 2 else SIG))
                        acts.append(a)
                i_a, f_a, g_a, o_a = acts
                ig = ws.tile([P, HT], F32, tag="ig", name=f"ig_{t}")
                nc.vector.tensor_mul(ig, i_a, g_a)
                if t == 0:
                    nc.vector.tensor_copy(out=cT, in_=ig)
                else:
                    fc = ws.tile([P, HT], F32, tag="fc", name=f"fc_{t}")
                    nc.vector.tensor_mul(fc, f_a, cT)
                    nc.vector.tensor_add(cT, ig, fc)
                tc_t = ws.tile([P, HT], F32, tag="tanhc", name=f"tanhc_{t}")
                nc.scalar.activation(out=tc_t, in_=cT, func=TANH)
                h_new = h16pool.tile([P, HT], F16, tag="h16", name=f"h16_{t}")
                nc.vector.tensor_mul(h_new, o_a, tc_t)
                h_prev = h_new

            # ---- head: pred = hT.T @ lin_wT + lin_b ----
            psP = psmm.tile([BLOC, 1], F32, tag="psL1", name="psP", bufs=2)
            for k in range(KH):
                nc.tensor.matmul(psP, lhsT=h_prev[:, k * BLOC:(k + 1) * BLOC],
                                 rhs=linw_sb[:, k:k + 1],
                                 start=(k == 0), stop=(k == KH - 1))
            pred_sb = const.tile([BLOC, 1], F32)
            nc.scalar.activation(out=pred_sb, in_=psP, func=IDENT,
                                 bias=linb_sb, scale=1.0)
            nc.sync.dma_start(out=d_pred[:], in_=pred_sb)

    nc.compile()
    _CACHE["nc"] = nc
    return nc


def _prep_in_maps(v, t, W_down, Wt_up_w, Wt_up_b, W_vt, W_ih, W_hh,
                  b_ih, b_hh, lin_w, lin_b):
    """Host-side shard/layout/dtype prep. Layout + cast only, no math."""
    WdT = np.zeros((NPAD, HIN), np.float16)
    WdT[:N] = np.ascontiguousarray(W_down.T).astype(np.float16)
    WvtT = np.ascontiguousarray(W_vt.T).astype(np.float16)
    WihT = np.ascontiguousarray(W_ih.T).astype(np.float16)
    WhhT = np.ascontiguousarray(W_hh.T).astype(np.float16)
    wt_row = np.ascontiguousarray(Wt_up_w.reshape(1, HT)).astype(np.float32)
    wtb_row = np.ascontiguousarray(Wt_up_b.reshape(1, HT)).astype(np.float32)
    bias_g = np.ascontiguousarray(
        (b_ih + b_hh).astype(np.float32).reshape(MC, P).T)
    lin_wT = np.ascontiguousarray(
        lin_w.reshape(KH, P).T).astype(np.float16)
    lin_b_col = np.full((BLOC, 1), np.float32(lin_b[0]), np.float32)

    shared = dict(WdT=WdT, WvtT=WvtT, WihT=WihT, WhhT=WhhT, wt_row=wt_row,
                  wtb_row=wtb_row, bias_g=bias_g, lin_wT=lin_wT,
                  lin_b_col=lin_b_col)
    in_maps = []
    for c in range(NCORES):
        b0 = c * BLOC
        # rows r = l*16 + b  (l-major)
        vr = v[b0:b0 + BLOC].transpose(1, 0, 2).reshape(R, N)
        vT = np.zeros((NPAD, R), np.float16)
        vT[:N] = vr.T.astype(np.float16)
        t_row = np.ascontiguousarray(
            t[b0:b0 + BLOC].T.reshape(1, R)).astype(np.float32)
        in_maps.append(dict(vT=vT, t_row=t_row, **shared))
    return in_maps


def kernel(**inputs):
    nc = _build()
    in_maps = _prep_in_maps(**inputs)
    res = run_bass_kernel_spmd(nc, in_maps, core_ids=list(range(NCORES)))
    return np.concatenate([res.results[c]["pred"] for c in range(NCORES)],
                          axis=0).astype(np.float32)
